# revision 35
# baseline (speedup 1.0000x reference)
"""MoE GPT-OSS experts kernel for 8x TRN2 NeuronCores (expert-parallel).

Strategy:
  - 8 experts, 8 cores: expert e -> core e.
  - Host computes the routing mask, gathers each expert's tokens into a
    padded capacity buffer (capacity = max tokens routed to any expert,
    rounded up), and pre-arranges all tensors in the exact SBUF layout the
    device consumes (so every DMA is contiguous).
  - Device computes, per expert, in the transposed layout (tokens on the
    matmul free dim, features on partitions):
        gateT/upT = W_{g,u}^T-chunks (stationary) @ xT (moving)   [I, T]
        act = (clip(up + bu) + 1) * gasig(min(gate + bg, LIMIT))  [I, T]
        outT = Wd-chunks (stationary) @ act (moving)              [H, T]
    where gasig(z) = z * sigmoid(1.702 z) (hardware Gelu_apprx_sigmoid).
  - Host applies per-(token, expert) routing weights, scatter-adds the
    expert outputs, and adds the rank-1 down-bias term w_eff @ bias_d.
    (The down bias commutes with the routing weighting, so the device
    never needs it.)

Matmuls run in bf16 (fp32 PSUM accumulation).
"""

import sys

if "/opt/trn_rl_repo" not in sys.path:
    sys.path.insert(0, "/opt/trn_rl_repo")

import numpy as np
import ml_dtypes

ALPHA = 1.702
LIMIT = 7.0
P = 128
H = 1024
I = 2048
E = 8
NCORES = 8
KO = H // P  # 8  k-chunks for gate/up matmul (contract over H)
KI = I // P  # 16 k-chunks for down matmul (contract over I)
MI = I // P  # 16 output chunks over I
MH = H // P  # 8  output chunks over H
MAX_N = 512  # PSUM bank: 512 fp32 per partition
N_WARMUP = 13  # dummy PE warmup matmuls

BF16 = ml_dtypes.bfloat16

_NC_CACHE: dict[int, object] = {}


def _build_nc(cap: int):
    """Build the Bass program for a given token capacity per expert."""
    import concourse.mybir as mybir
    import concourse.tile as tile
    from concourse import bacc

    bf = mybir.dt.bfloat16
    f32 = mybir.dt.float32
    AF = mybir.ActivationFunctionType
    ALU = mybir.AluOpType

    class _LeanTC(tile.TileContext):
        def _drain_and_barrier(self, tick_clock, wait_clock):
            from concourse.vector_clock import ScopedClock

            drain_inst = self.nc.sync.drain()
            wait_clock.add_sem_waits(
                drain_inst.ins, ScopedClock({None: tick_clock.global_clock})
            )
            self.nc.all_engine_barrier()
            popped = self.nc._tile_sem_poison_stack.pop()
            assert popped is self._sem_poison
            self.nc.clear_and_free_semaphores(list(self.sems.allocated().values()))

    nc = bacc.Bacc()
    xT_d = nc.declare_dram_parameter("xT", [P, KO, cap], bf, isOutput=False)
    wg_d = nc.declare_dram_parameter("wg", [P, MI, KO, P], bf, isOutput=False)
    wu_d = nc.declare_dram_parameter("wu", [P, MI, KO, P], bf, isOutput=False)
    wd_d = nc.declare_dram_parameter("wd", [P, MH, KI, P], bf, isOutput=False)
    bg_d = nc.declare_dram_parameter("bg", [P, MI], f32, isOutput=False)
    bu_d = nc.declare_dram_parameter("bu", [P, MI], f32, isOutput=False)
    out_d = nc.declare_dram_parameter("outT", [H, cap], f32, isOutput=True)

    slices = [(off, min(MAX_N, cap - off)) for off in range(0, cap, MAX_N)]

    with _LeanTC(nc) as tc:
        with (
            tc.tile_pool(name="w", bufs=1) as wpool,
            tc.tile_pool(name="a", bufs=3) as apool,
            tc.tile_pool(name="o", bufs=3) as opool,
            tc.tile_pool(name="pgu", bufs=2, space="PSUM") as ppool,
            tc.tile_pool(name="pd", bufs=2, space="PSUM") as dpool,
            tc.tile_pool(name="pw", bufs=1, space="PSUM") as wmpool,
        ):
            # PE warmup: dummy matmuls with no DMA deps keep the PE busy
            # while input DMAs land, so HAM un-throttles before real work.
            warm_src = wpool.tile([P, 256], bf, tag="warm_src")
            nc.vector.memset(warm_src[:], 0)
            warm_ps = wmpool.tile([P, 256], f32, tag="warm_ps")
            for _ in range(N_WARMUP):
                nc.tensor.matmul(
                    warm_ps[:], warm_src[:, :P], warm_src[:], start=True, stop=True
                )

            # Persistent SBUF residents. dma_start issue costs ~0.6us on the
            # sync sequencer, so use few, large DMAs: the first gate/up
            # pair's weights small + early, the rest in big groups.
            GU_GROUPS = [(0, 1), (1, 2), (2, 4), (4, 6), (6, 9), (9, 12), (12, 16)]
            WD_GROUPS = [(0, 4), (4, 8)]
            wg_grp = []
            wu_grp = []
            for gi, (a, b) in enumerate(GU_GROUPS):
                wg_grp.append(wpool.tile([P, b - a, KO, P], bf, tag=f"wgg{gi}",
                                         name=f"wgg{gi}"))
                wu_grp.append(wpool.tile([P, b - a, KO, P], bf, tag=f"wug{gi}",
                                         name=f"wug{gi}"))

            def _lookup(grps, groups, i):
                for gi, (a, b) in enumerate(groups):
                    if a <= i < b:
                        return grps[gi][:, i - a]
                raise AssertionError(i)

            nc.sync.dma_start(wg_grp[0][:], wg_d[:, 0:1])
            xT_lo = wpool.tile([P, KO // 2, cap], bf, tag="xT_lo")
            nc.sync.dma_start(xT_lo[:], xT_d[:, : KO // 2])
            nc.sync.dma_start(wu_grp[0][:], wu_d[:, 0:1])
            xT_hi = wpool.tile([P, KO // 2, cap], bf, tag="xT_hi")
            nc.sync.dma_start(xT_hi[:], xT_d[:, KO // 2 :])
            xT_sb = [xT_lo[:, k] for k in range(KO // 2)] + [
                xT_hi[:, k] for k in range(KO // 2)
            ]
            nc.sync.dma_start(wg_grp[1][:], wg_d[:, 1:2])
            nc.sync.dma_start(wu_grp[1][:], wu_d[:, 1:2])
            bg = wpool.tile([P, MI], f32, tag="bg")
            nc.sync.dma_start(bg[:], bg_d[:])
            bu = wpool.tile([P, MI], f32, tag="bu")
            nc.sync.dma_start(bu[:], bu_d[:])
            for gi, (a, b) in list(enumerate(GU_GROUPS))[2:]:
                nc.sync.dma_start(wg_grp[gi][:], wg_d[:, a:b])
                nc.sync.dma_start(wu_grp[gi][:], wu_d[:, a:b])
            wd_grp = []
            for gi, (a, b) in enumerate(WD_GROUPS):
                wd_grp.append(wpool.tile([P, b - a, KI, P], bf, tag=f"wdg{gi}",
                                         name=f"wdg{gi}"))
                nc.sync.dma_start(wd_grp[gi][:], wd_d[:, a:b])

            wg_sb = [_lookup(wg_grp, GU_GROUPS, m) for m in range(MI)]
            wu_sb = [_lookup(wu_grp, GU_GROUPS, m) for m in range(MI)]
            wd_sb = [_lookup(wd_grp, WD_GROUPS, h) for h in range(MH)]
            act_sb = [wpool.tile([P, cap], bf, tag=f"act{m}", name=f"act{m}")
                      for m in range(MI)]

            # Phase 1: gate/up matmuls + GEGLU activation.
            for off, n in slices:
                for m in range(MI):
                    pg = ppool.tile([P, MAX_N], f32, tag="pg", name="pg")[:, :n]
                    pu = ppool.tile([P, MAX_N], f32, tag="pu", name="pu")[:, :n]
                    for k in range(KO):
                        nc.tensor.matmul(
                            pg,
                            wg_sb[m][:, k],
                            xT_sb[k][:, off : off + n],
                            start=(k == 0),
                            stop=(k == KO - 1),
                        )
                    for k in range(KO):
                        nc.tensor.matmul(
                            pu,
                            wu_sb[m][:, k],
                            xT_sb[k][:, off : off + n],
                            start=(k == 0),
                            stop=(k == KO - 1),
                        )
                    gp = apool.tile([P, MAX_N], f32, tag="gp", name="gp")[:, :n]
                    nc.vector.tensor_scalar(
                        gp, pg, bg[:, m : m + 1], LIMIT, ALU.add, ALU.min
                    )
                    glu = apool.tile([P, MAX_N], f32, tag="glu", name="glu")[:, :n]
                    nc.scalar.activation(glu, gp, AF.Gelu_apprx_sigmoid)
                    u2 = apool.tile([P, MAX_N], f32, tag="u2", name="u2")[:, :n]
                    nc.vector.tensor_scalar(
                        u2, pu, bu[:, m : m + 1], LIMIT, ALU.add, ALU.min
                    )
                    nc.vector.tensor_scalar(u2, u2, -LIMIT, 1.0, ALU.max, ALU.add)
                    nc.vector.tensor_mul(act_sb[m][:, off : off + n], u2, glu)

            # Phase 2: down matmuls; PSUM staged through SBUF, then DMA out.
            for off, n in slices:
                for h in range(MH):
                    po = dpool.tile([P, MAX_N], f32, tag="po", name="po")[:, :n]
                    for k in range(KI):
                        nc.tensor.matmul(
                            po,
                            wd_sb[h][:, k],
                            act_sb[k][:, off : off + n],
                            start=(k == 0),
                            stop=(k == KI - 1),
                        )
                    ot = opool.tile([P, MAX_N], f32, tag="ot", name="ot")[:, :n]
                    nc.vector.tensor_copy(ot, po)
                    nc.sync.dma_start(out_d[h * P : (h + 1) * P, off : off + n], ot)

    nc.finalize()
    return nc


def _prep_inputs(hidden_states, router_indices, routing_weights,
                 gate_up_proj, gate_up_proj_bias, down_proj):
    """Host-side routing + layout shuffling. Returns (in_maps, meta)."""
    x = np.ascontiguousarray(np.asarray(hidden_states, dtype=np.float32)).reshape(-1, H)
    T = x.shape[0]
    ri = np.asarray(router_indices).astype(np.int64).reshape(T, -1)
    rw = np.asarray(routing_weights, dtype=np.float32).reshape(T, E)

    sel = np.zeros((T, E), dtype=bool)
    sel[np.arange(T)[:, None], ri] = True
    w_eff = rw * sel

    idx_per_e = [np.nonzero(sel[:, e])[0] for e in range(E)]
    counts = np.array([len(ix) for ix in idx_per_e])
    cap = int(max(P, -(-int(counts.max()) // 4) * 4))

    gu = np.asarray(gate_up_proj, dtype=np.float32)
    gub = np.asarray(gate_up_proj_bias, dtype=np.float32)
    dn = np.asarray(down_proj, dtype=np.float32)

    in_maps = []
    for e in range(E):
        xg = np.zeros((cap, H), dtype=np.float32)
        xg[: counts[e]] = x[idx_per_e[e]]
        xT = np.ascontiguousarray(
            xg.T.reshape(KO, P, cap).transpose(1, 0, 2)
        ).astype(BF16)
        wg = np.ascontiguousarray(
            gu[e][:, 0::2].reshape(KO, P, MI, P).transpose(1, 2, 0, 3)
        ).astype(BF16)
        wu = np.ascontiguousarray(
            gu[e][:, 1::2].reshape(KO, P, MI, P).transpose(1, 2, 0, 3)
        ).astype(BF16)
        wd = np.ascontiguousarray(
            dn[e].reshape(KI, P, MH, P).transpose(1, 2, 0, 3)
        ).astype(BF16)
        bg = np.ascontiguousarray(gub[e][0::2].reshape(MI, P).T).astype(np.float32)
        bu = np.ascontiguousarray(gub[e][1::2].reshape(MI, P).T).astype(np.float32)
        in_maps.append({"xT": xT, "wg": wg, "wu": wu, "wd": wd, "bg": bg, "bu": bu})

    return in_maps, (w_eff, idx_per_e, counts, cap, T)


def _run(inputs: dict, trace: bool = False):
    from concourse.bass_utils import run_bass_kernel_spmd

    in_maps, (w_eff, idx_per_e, counts, cap, T) = _prep_inputs(
        inputs["hidden_states"], inputs["router_indices"],
        inputs["routing_weights"], inputs["gate_up_proj"],
        inputs["gate_up_proj_bias"], inputs["down_proj"],
    )

    if cap not in _NC_CACHE:
        _NC_CACHE[cap] = _build_nc(cap)
    nc = _NC_CACHE[cap]

    res = run_bass_kernel_spmd(nc, in_maps, core_ids=list(range(NCORES)), trace=trace)

    dnb = np.asarray(inputs["down_proj_bias"], dtype=np.float32)
    y = w_eff @ dnb  # rank-1-per-expert down-bias term, [T, H]
    for e in range(E):
        cnt = counts[e]
        if cnt == 0:
            continue
        idx = idx_per_e[e]
        outT = res.results[e]["outT"]  # [H, cap] f32
        y[idx] += outT[:, :cnt].T * w_eff[idx, e][:, None]

    hs = np.asarray(inputs["hidden_states"])
    return y.reshape(hs.shape).astype(np.float32), res


def kernel(**inputs) -> np.ndarray:
    out, _ = _run(inputs, trace=False)
    return out


# revision 36
# speedup vs baseline: 1.0411x; 1.0411x over previous
"""MoE GPT-OSS experts kernel for 8x TRN2 NeuronCores (expert-parallel).

Strategy:
  - 8 experts, 8 cores: expert e -> core e.
  - Host computes the routing mask, gathers each expert's tokens into a
    padded capacity buffer (capacity = max tokens routed to any expert,
    rounded up), and pre-arranges all tensors in the exact SBUF layout the
    device consumes (so every DMA is contiguous).
  - Device computes, per expert, in the transposed layout (tokens on the
    matmul free dim, features on partitions):
        gateT/upT = W_{g,u}^T-chunks (stationary) @ xT (moving)   [I, T]
        act = (clip(up + bu) + 1) * gasig(min(gate + bg, LIMIT))  [I, T]
        outT = Wd-chunks (stationary) @ act (moving)              [H, T]
    where gasig(z) = z * sigmoid(1.702 z) (hardware Gelu_apprx_sigmoid).
  - Host applies per-(token, expert) routing weights, scatter-adds the
    expert outputs, and adds the rank-1 down-bias term w_eff @ bias_d.
    (The down bias commutes with the routing weighting, so the device
    never needs it.)

Matmuls run in bf16 (fp32 PSUM accumulation).
"""

import sys

if "/opt/trn_rl_repo" not in sys.path:
    sys.path.insert(0, "/opt/trn_rl_repo")

import numpy as np
import ml_dtypes

ALPHA = 1.702
LIMIT = 7.0
P = 128
H = 1024
I = 2048
E = 8
NCORES = 8
KO = H // P  # 8  k-chunks for gate/up matmul (contract over H)
KI = I // P  # 16 k-chunks for down matmul (contract over I)
MI = I // P  # 16 output chunks over I
MH = H // P  # 8  output chunks over H
MAX_N = 512  # PSUM bank: 512 fp32 per partition
N_WARMUP = 24  # dummy PE warmup matmuls

BF16 = ml_dtypes.bfloat16

_NC_CACHE: dict[int, object] = {}


def _build_nc(cap: int):
    """Build the Bass program for a given token capacity per expert."""
    import concourse.mybir as mybir
    import concourse.tile as tile
    from concourse import bacc

    bf = mybir.dt.bfloat16
    f32 = mybir.dt.float32
    AF = mybir.ActivationFunctionType
    ALU = mybir.AluOpType

    class _LeanTC(tile.TileContext):
        def _drain_and_barrier(self, tick_clock, wait_clock):
            from concourse.vector_clock import ScopedClock

            drain_inst = self.nc.sync.drain()
            wait_clock.add_sem_waits(
                drain_inst.ins, ScopedClock({None: tick_clock.global_clock})
            )
            self.nc.all_engine_barrier()
            popped = self.nc._tile_sem_poison_stack.pop()
            assert popped is self._sem_poison
            self.nc.clear_and_free_semaphores(list(self.sems.allocated().values()))

    nc = bacc.Bacc()
    xT_d = nc.declare_dram_parameter("xT", [P, KO, cap], bf, isOutput=False)
    wg_d = nc.declare_dram_parameter("wg", [P, MI, KO, P], bf, isOutput=False)
    wu_d = nc.declare_dram_parameter("wu", [P, MI, KO, P], bf, isOutput=False)
    wd_d = nc.declare_dram_parameter("wd", [P, MH, KI, P], bf, isOutput=False)
    bg_d = nc.declare_dram_parameter("bg", [P, MI], f32, isOutput=False)
    bu_d = nc.declare_dram_parameter("bu", [P, MI], f32, isOutput=False)
    out_d = nc.declare_dram_parameter("outT", [H, cap], f32, isOutput=True)

    slices = [(off, min(MAX_N, cap - off)) for off in range(0, cap, MAX_N)]

    with _LeanTC(nc) as tc:
        with (
            tc.tile_pool(name="w", bufs=1) as wpool,
            tc.tile_pool(name="a", bufs=3) as apool,
            tc.tile_pool(name="o", bufs=3) as opool,
            tc.tile_pool(name="pgu", bufs=2, space="PSUM") as ppool,
            tc.tile_pool(name="pd", bufs=2, space="PSUM") as dpool,
            tc.tile_pool(name="pw", bufs=1, space="PSUM") as wmpool,
        ):
            # PE warmup: dummy matmuls with no DMA deps keep the PE busy
            # while input DMAs land, so HAM un-throttles before real work.
            warm_src = wpool.tile([P, 256], bf, tag="warm_src")
            nc.vector.memset(warm_src[:], 0)
            warm_ps = wmpool.tile([P, 256], f32, tag="warm_ps")
            for _ in range(N_WARMUP):
                nc.tensor.matmul(
                    warm_ps[:], warm_src[:, :P], warm_src[:], start=True, stop=True
                )

            # Persistent SBUF residents. dma_start issue costs ~0.6us on the
            # sync sequencer, so use few, large DMAs: the first gate/up
            # pair's weights small + early, the rest in big groups.
            GU_GROUPS = [(0, 1), (1, 2), (2, 4), (4, 6), (6, 9), (9, 12), (12, 16)]
            WD_GROUPS = [(0, 4), (4, 8)]
            wg_grp = []
            wu_grp = []
            for gi, (a, b) in enumerate(GU_GROUPS):
                wg_grp.append(wpool.tile([P, b - a, KO, P], bf, tag=f"wgg{gi}",
                                         name=f"wgg{gi}"))
                wu_grp.append(wpool.tile([P, b - a, KO, P], bf, tag=f"wug{gi}",
                                         name=f"wug{gi}"))

            def _lookup(grps, groups, i):
                for gi, (a, b) in enumerate(groups):
                    if a <= i < b:
                        return grps[gi][:, i - a]
                raise AssertionError(i)

            nc.sync.dma_start(wg_grp[0][:], wg_d[:, 0:1])
            xT_lo = wpool.tile([P, KO // 2, cap], bf, tag="xT_lo")
            nc.sync.dma_start(xT_lo[:], xT_d[:, : KO // 2])
            nc.sync.dma_start(wu_grp[0][:], wu_d[:, 0:1])
            xT_hi = wpool.tile([P, KO // 2, cap], bf, tag="xT_hi")
            nc.sync.dma_start(xT_hi[:], xT_d[:, KO // 2 :])
            xT_sb = [xT_lo[:, k] for k in range(KO // 2)] + [
                xT_hi[:, k] for k in range(KO // 2)
            ]
            nc.sync.dma_start(wg_grp[1][:], wg_d[:, 1:2])
            nc.sync.dma_start(wu_grp[1][:], wu_d[:, 1:2])
            bg = wpool.tile([P, MI], f32, tag="bg")
            nc.sync.dma_start(bg[:], bg_d[:])
            bu = wpool.tile([P, MI], f32, tag="bu")
            nc.sync.dma_start(bu[:], bu_d[:])
            for gi, (a, b) in list(enumerate(GU_GROUPS))[2:]:
                nc.sync.dma_start(wg_grp[gi][:], wg_d[:, a:b])
                nc.sync.dma_start(wu_grp[gi][:], wu_d[:, a:b])
            wd_grp = []
            for gi, (a, b) in enumerate(WD_GROUPS):
                wd_grp.append(wpool.tile([P, b - a, KI, P], bf, tag=f"wdg{gi}",
                                         name=f"wdg{gi}"))
                nc.sync.dma_start(wd_grp[gi][:], wd_d[:, a:b])

            wg_sb = [_lookup(wg_grp, GU_GROUPS, m) for m in range(MI)]
            wu_sb = [_lookup(wu_grp, GU_GROUPS, m) for m in range(MI)]
            wd_sb = [_lookup(wd_grp, WD_GROUPS, h) for h in range(MH)]
            act_sb = [wpool.tile([P, cap], bf, tag=f"act{m}", name=f"act{m}")
                      for m in range(MI)]

            # Phase 1: gate/up matmuls + GEGLU activation.
            for off, n in slices:
                for m in range(MI):
                    pg = ppool.tile([P, MAX_N], f32, tag="pg", name="pg")[:, :n]
                    pu = ppool.tile([P, MAX_N], f32, tag="pu", name="pu")[:, :n]
                    for k in range(KO):
                        nc.tensor.matmul(
                            pg,
                            wg_sb[m][:, k],
                            xT_sb[k][:, off : off + n],
                            start=(k == 0),
                            stop=(k == KO - 1),
                        )
                    for k in range(KO):
                        nc.tensor.matmul(
                            pu,
                            wu_sb[m][:, k],
                            xT_sb[k][:, off : off + n],
                            start=(k == 0),
                            stop=(k == KO - 1),
                        )
                    gp = apool.tile([P, MAX_N], f32, tag="gp", name="gp")[:, :n]
                    nc.vector.tensor_scalar(
                        gp, pg, bg[:, m : m + 1], LIMIT, ALU.add, ALU.min
                    )
                    glu = apool.tile([P, MAX_N], f32, tag="glu", name="glu")[:, :n]
                    nc.scalar.activation(glu, gp, AF.Gelu_apprx_sigmoid)
                    u2 = apool.tile([P, MAX_N], f32, tag="u2", name="u2")[:, :n]
                    nc.vector.tensor_scalar(
                        u2, pu, bu[:, m : m + 1], LIMIT, ALU.add, ALU.min
                    )
                    nc.vector.tensor_scalar(u2, u2, -LIMIT, 1.0, ALU.max, ALU.add)
                    nc.vector.tensor_mul(act_sb[m][:, off : off + n], u2, glu)

            # Phase 2: down matmuls; PSUM staged through SBUF, then DMA out.
            for off, n in slices:
                for h in range(MH):
                    po = dpool.tile([P, MAX_N], f32, tag="po", name="po")[:, :n]
                    for k in range(KI):
                        nc.tensor.matmul(
                            po,
                            wd_sb[h][:, k],
                            act_sb[k][:, off : off + n],
                            start=(k == 0),
                            stop=(k == KI - 1),
                        )
                    ot = opool.tile([P, MAX_N], f32, tag="ot", name="ot")[:, :n]
                    nc.vector.tensor_copy(ot, po)
                    nc.sync.dma_start(out_d[h * P : (h + 1) * P, off : off + n], ot)

    nc.finalize()
    return nc


def _prep_inputs(hidden_states, router_indices, routing_weights,
                 gate_up_proj, gate_up_proj_bias, down_proj):
    """Host-side routing + layout shuffling. Returns (in_maps, meta)."""
    x = np.ascontiguousarray(np.asarray(hidden_states, dtype=np.float32)).reshape(-1, H)
    T = x.shape[0]
    ri = np.asarray(router_indices).astype(np.int64).reshape(T, -1)
    rw = np.asarray(routing_weights, dtype=np.float32).reshape(T, E)

    sel = np.zeros((T, E), dtype=bool)
    sel[np.arange(T)[:, None], ri] = True
    w_eff = rw * sel

    idx_per_e = [np.nonzero(sel[:, e])[0] for e in range(E)]
    counts = np.array([len(ix) for ix in idx_per_e])
    cap = int(max(P, -(-int(counts.max()) // 4) * 4))

    gu = np.asarray(gate_up_proj, dtype=np.float32)
    gub = np.asarray(gate_up_proj_bias, dtype=np.float32)
    dn = np.asarray(down_proj, dtype=np.float32)

    in_maps = []
    for e in range(E):
        xg = np.zeros((cap, H), dtype=np.float32)
        xg[: counts[e]] = x[idx_per_e[e]]
        xT = np.ascontiguousarray(
            xg.T.reshape(KO, P, cap).transpose(1, 0, 2)
        ).astype(BF16)
        wg = np.ascontiguousarray(
            gu[e][:, 0::2].reshape(KO, P, MI, P).transpose(1, 2, 0, 3)
        ).astype(BF16)
        wu = np.ascontiguousarray(
            gu[e][:, 1::2].reshape(KO, P, MI, P).transpose(1, 2, 0, 3)
        ).astype(BF16)
        wd = np.ascontiguousarray(
            dn[e].reshape(KI, P, MH, P).transpose(1, 2, 0, 3)
        ).astype(BF16)
        bg = np.ascontiguousarray(gub[e][0::2].reshape(MI, P).T).astype(np.float32)
        bu = np.ascontiguousarray(gub[e][1::2].reshape(MI, P).T).astype(np.float32)
        in_maps.append({"xT": xT, "wg": wg, "wu": wu, "wd": wd, "bg": bg, "bu": bu})

    return in_maps, (w_eff, idx_per_e, counts, cap, T)


def _run(inputs: dict, trace: bool = False):
    from concourse.bass_utils import run_bass_kernel_spmd

    in_maps, (w_eff, idx_per_e, counts, cap, T) = _prep_inputs(
        inputs["hidden_states"], inputs["router_indices"],
        inputs["routing_weights"], inputs["gate_up_proj"],
        inputs["gate_up_proj_bias"], inputs["down_proj"],
    )

    if cap not in _NC_CACHE:
        _NC_CACHE[cap] = _build_nc(cap)
    nc = _NC_CACHE[cap]

    res = run_bass_kernel_spmd(nc, in_maps, core_ids=list(range(NCORES)), trace=trace)

    dnb = np.asarray(inputs["down_proj_bias"], dtype=np.float32)
    y = w_eff @ dnb  # rank-1-per-expert down-bias term, [T, H]
    for e in range(E):
        cnt = counts[e]
        if cnt == 0:
            continue
        idx = idx_per_e[e]
        outT = res.results[e]["outT"]  # [H, cap] f32
        y[idx] += outT[:, :cnt].T * w_eff[idx, e][:, None]

    hs = np.asarray(inputs["hidden_states"])
    return y.reshape(hs.shape).astype(np.float32), res


def kernel(**inputs) -> np.ndarray:
    out, _ = _run(inputs, trace=False)
    return out
